# revision 55
# baseline (speedup 1.0000x reference)
"""Enframe kernel for Trainium2 (Bass/Tile), 8-core data parallel.

Problem: input (16, 480000) f32, frame_length=2048, hop=512.
  out[b, w, f] = input[b, w + 512*f],  f in [0, 934), w in [0, 2048).

Key identity: write w = 512*h + l (h in [0,4), l in [0,512)). Then
  out[b, 512*h + l, f] = input[b, 512*(f + h) + l] = in3[b, f + h, l]
where in3 = input[:, :937*512].reshape(B, 937, 512). So the whole op is ONE
(937, 512) -> (512, 937) transpose per clip; the four h-blocks of the output
are shifted overlapping windows T[:, h : h+934] of that transpose.

Shipped default "b9um" (= "b9" fold kernel with psum_bufs=2/psum2_bufs=6,
t2_bufs=3, and the four h-stores merged into ONE DMA per clip from a
single [128, 4, 4, F] tile — same 7472 B descriptors, fewer instruction
boundaries, clip 0 on the SP ring / clip 1 on ACT; ~1-2 us faster than
per-h stores in paired in-process runs). ~22-35 us/iter depending on
machine state, vs ~62 us for the old f32 "v1Lt" and ~37-42 us for bf16
"b1". The op is memory-bound and the b9 family sits at the measured
pure-DMA floor of its byte volume:

  1. bf16 end-to-end on device (host converts f32->bf16 input and
     bf16->f32 output; only error is the input round, max rel ~2e-3 vs
     the 2e-2 gate). Halves DMA traffic: 19.1 -> 9.57 MB per core.
  2. Pass 1: v1-style 128x128 TensorE transposes (contiguous bf16 lhsT)
     -> PSUM, ACT-copy into t_t[p, c, g] = T[128c + p, g].
  3. Pass 2 ("fold"): 0/1 selection matrices S[j,c] via a second PE pass
     accumulate ps2_j[q, g] = T[4q + j, g] (exactly one c contributes per
     element). DVE-copies the four h-shifted windows into per-h tiles
     t2[h][q, j, f] = out[b, 512h + 4q + j, f].
  4. Stores: per (clip, h), DRAM AP "(q j) f -> q (j f)" makes every
     descriptor a contiguous 4-row 7472 B run; stores alternate SP/ACT
     rings.

Measured DMA facts (in-process probes, this part): aggregate per-core DMA
byte rate is the cap (~300 GB/s at 7472 B descriptors, ~185 GB/s at
1868 B); splitting rings does NOT scale it; bigger descriptors do. The
dma-only floor for this byte volume is ~30.7 us and b9 matches it, i.e.
the PE fold pass and DVE copies are fully hidden.

Why pass 2 exists: merging 4 consecutive DRAM rows per descriptor needs
out rows 4q..4q+3 on partition q, which is a partition-space permute that
only PE (or DMA) can do. The previous session's "v8" did it with
strided-free-dim permute copies on ACT/DVE and crashed sporadically
(NRT_EXEC_UNIT_UNRECOVERABLE, 4 of ~30 fresh processes); strided-free-dim
f32 lhsT in a transpose matmul crashes deterministically. b9 has NO
strided reads: the fold is standard contiguous matmuls. Verified stable
across 12 fresh processes (plus the bench/chk runs, ~1M+ rep executions)
with bit-identical results.

Known-fatal on this HW (do not reintroduce): strided-free-dim lhsT in
transpose matmuls; gpsimd tensor_copy PSUM->SBUF fails to compile
(walrus_driver). A sporadic NRT_EXEC_UNIT_UNRECOVERABLE hit a fresh
process running "b9ug" (t_bufs=3) / "b9uh" (pass-1 copies on DVE) — the
only crash all session; neither beat b9ud, so they are not shipped and
should be treated as suspect.
"""

import numpy as np

N_CORES = 8
BATCH = 16
B = BATCH // N_CORES  # clips per core
S = 480000
FRAME = 2048
HOP = 512
F = (S - FRAME) // HOP + 1  # 934
G = FRAME // HOP + F - 1  # 937 distinct 512-sample rows used
G_FULL = G // 128  # 7 full partition chunks
G_TAIL = G - 128 * G_FULL  # 41
H = FRAME // HOP  # 4 output row-blocks of 512

_CACHE: dict = {}


_VARIANTS = {
    # store_mode: "merged" (4 stores/clip, 1.9 MB, p-major enumeration) or
    #             "per_c" (16 stores/clip, 478 KB, sequential DRAM)
    # split_io: cut loads/stores at the psum-half boundary for earlier starts
    "v1": dict(store_mode="merged", split_io=False, bufs=2, psum_bufs=4),
    "v1p": dict(store_mode="merged", split_io=False, bufs=2, psum_bufs=8),
    # split only the loads (not stores): earlier transpose start, same stores
    "v1L": dict(store_mode="merged", split_io=False, split_loads=True, bufs=2, psum_bufs=4),
    # v1L with a 3rd T buffer: decouple copies from store-slot release
    "v1Lt": dict(store_mode="merged", split_io=False, split_loads=True, bufs=2, t_bufs=3, psum_bufs=4),
    # v1Lt with a 4th T buffer
    "v1Lt4": dict(store_mode="merged", split_io=False, split_loads=True, bufs=2, t_bufs=4, psum_bufs=4),
    # v1Lt plus a 3rd A buffer as well
    "v1Lta": dict(store_mode="merged", split_io=False, split_loads=True, bufs=3, t_bufs=3, psum_bufs=4),
    "v2": dict(store_mode="merged", split_io=True, bufs=2, psum_bufs=8),
    "v3": dict(store_mode="per_c", split_io=False, bufs=2, psum_bufs=4),
    "v4": dict(store_mode="merged", split_io=False, bufs=3, psum_bufs=8),
    "v5": dict(store_mode="per_c", split_io=False, bufs=3, psum_bufs=8),
    # ring balance: n of the 8 stores go to the ACT (scalar) ring alongside
    # the loads, to even out bytes between the two HWDGE rings
    "v6": dict(
        store_mode="merged", split_io=False, bufs=2, psum_bufs=4, act_stores=3
    ),
    "v7": dict(
        store_mode="merged", split_io=False, bufs=2, psum_bufs=4, act_stores=2
    ),
    # timing-only: same DMAs, no transpose/copies — measures the pure DMA
    # ceiling of this access pattern (output is garbage)
    "dma": dict(
        store_mode="merged", split_io=False, bufs=2, psum_bufs=4, dma_only=True
    ),
    # dma-only with only half the stores: separates bytes-bound from
    # overhead-bound
    "dma2": dict(
        store_mode="merged",
        split_io=False,
        bufs=2,
        psum_bufs=4,
        dma_only=True,
        store_hs=(0, 1),
    ),
    # dma-only, same bytes but idealized stores: 14992 B descriptors into
    # fully linear DRAM — probes whether descriptor size lifts write BW
    "dma3": dict(
        store_mode="linear", split_io=False, bufs=2, psum_bufs=4, dma_only=True
    ),
    # interleaved partition mapping: output row l = 4q + j lives on partition
    # q, T tiles are per-h [128, 4, 934] so (j, f) merge into one contiguous
    # 3736-element run -> real 14944 B store descriptors
    # final: interleaved partition mapping with contiguous lhsT via ACT
    # pre-permute. NOTE: adding act_stores or split_io here caused
    # NRT_EXEC_UNIT_UNRECOVERABLE crashes (as "v9") — do not re-add.
    "v8": dict(store_mode="interleaved", split_io=False, bufs=2, psum_bufs=4),
    "v8p": dict(store_mode="interleaved", split_io=False, bufs=2, psum_bufs=8),
    # like v8p but the column pre-permute runs on DVE instead of ACT — the
    # ACT-copy version crashed sporadically (NRT_EXEC_UNIT_UNRECOVERABLE)
    "v8d": dict(
        store_mode="interleaved",
        split_io=False,
        bufs=2,
        psum_bufs=8,
        dve_permute=True,
    ),
    # ---- bf16 family: device computes/stores bf16 (host converts back to
    # f32; exact except the input f32->bf16 round, max rel err ~2e-3 vs the
    # 2e-2 gate). Halves the dominant store traffic: 19.1 MB -> 9.6 MB/core.
    "b1": dict(
        store_mode="merged", split_io=False, split_loads=True, bufs=2,
        t_bufs=3, psum_bufs=4, bf16=True,
    ),
    # ring balance for the new byte mix (stores 7.65 MB vs loads 1.92 MB):
    # move n of the 8 stores onto the ACT ring
    "b6": dict(
        store_mode="merged", split_io=False, split_loads=True, bufs=2,
        t_bufs=3, psum_bufs=4, bf16=True, act_stores=3,
    ),
    "b7": dict(
        store_mode="merged", split_io=False, split_loads=True, bufs=2,
        t_bufs=3, psum_bufs=4, bf16=True, act_stores=2,
    ),
    # dma-only bf16 ceiling probe (output garbage)
    "bdma": dict(
        store_mode="merged", split_io=False, bufs=2, psum_bufs=4, bf16=True,
        dma_only=True,
    ),
    # more buffering: decouple copies/stores further
    "b1t4": dict(
        store_mode="merged", split_io=False, split_loads=True, bufs=2,
        t_bufs=4, psum_bufs=8, bf16=True,
    ),
    # split the PSUM->SBUF copies across engines (per c-block):
    # v=DVE, a=ACT, p=Pool
    "b1c": dict(
        store_mode="merged", split_io=False, split_loads=True, bufs=2,
        t_bufs=3, psum_bufs=4, bf16=True, copy_engines="vvaa",
    ),
    # NOTE: gpsimd tensor_copy PSUM->SBUF fails to lower (walrus_driver
    # crash), so no "p" engines in copy_engines.
    # dma-only probes: bdma2 halves store bytes, bdma3 uses idealized linear
    # 7472 B descriptors — separates bytes-bound from descriptor-bound
    "bdma2": dict(
        store_mode="merged", split_io=False, bufs=2, psum_bufs=4, bf16=True,
        dma_only=True, store_hs=(0, 1),
    ),
    "bdma3": dict(
        store_mode="linear", split_io=False, bufs=2, psum_bufs=4, bf16=True,
        dma_only=True,
    ),
    # ring-scaling probes: same stores split 50/50 across SP + ACT rings
    "bdma6": dict(
        store_mode="merged", split_io=False, bufs=2, psum_bufs=4, bf16=True,
        dma_only=True, act_stores=4,
    ),
    "bdma36": dict(
        store_mode="linear", split_io=False, bufs=2, psum_bufs=4, bf16=True,
        dma_only=True, act_stores=4,
    ),
    # fold: real 7472 B store descriptors (out rows 4q..4q+3 on partition q)
    # via a SECOND PE pass with constant 0/1 fold matrices — no strided
    # reads anywhere (the v8 family's suspected crash trigger).
    "b9": dict(
        store_mode="fold", split_io=False, split_loads=True, bufs=2,
        t_bufs=2, t2_bufs=2, psum_bufs=4, psum2_bufs=4, bf16=True,
    ),
    # b9 with the pass-2 matmuls run twice (dummy group then real group into
    # the same PSUM tile; start=True resets, so output is unchanged) — probes
    # whether PE has slack for a 2x fold (the 8-row/14944 B design)
    "b9x2": dict(
        store_mode="fold", split_io=False, split_loads=True, bufs=2,
        t_bufs=2, t2_bufs=2, psum_bufs=4, psum2_bufs=4, bf16=True, pe_x2=True,
    ),
    # linear dma-only probe with 14944 B descriptors, stores split over rings
    "bdma4": dict(
        store_mode="linear2", split_io=False, bufs=2, psum_bufs=4, bf16=True,
        dma_only=True, act_stores=4,
    ),
    # 8-row fold: partition q holds out rows 8q..8q+7 of a 1024-row block ->
    # 14944 B store descriptors, 2 stores per clip. 2x the fold matmuls of
    # b9 (PE slack confirmed by b9x2), copies partition-sliced by q-half.
    # MEASURED 72-74 us: half-width copies double DVE serial time. Dead end.
    "b10": dict(
        store_mode="fold8", split_io=False, split_loads=True, bufs=2,
        t_bufs=2, t2_bufs=2, psum_bufs=4, psum2_bufs=4, bf16=True,
    ),
    # overlap-store folds: fold once into t[q, j, g] = T[kq+j, g]; the 4
    # h-stores read overlapping windows t[:, :, h:h+F] (duplication done by
    # the DMA's SBUF reads, not engine copies). DRAM-side runs stay 7472 B
    # (b9o) / 14944 B (b10o); probes whether descriptor efficiency follows
    # the DRAM side when SBUF runs are 1868 B.
    "b9o": dict(
        store_mode="fold", split_io=False, split_loads=True, bufs=2,
        t_bufs=2, t2_bufs=2, psum_bufs=4, psum2_bufs=4, bf16=True,
        overlap_store=True,
    ),
    "b10o": dict(
        store_mode="fold8", split_io=False, split_loads=True, bufs=2,
        t_bufs=2, t2_bufs=2, psum_bufs=4, psum2_bufs=4, bf16=True,
        overlap_store=True,
    ),
    # tfold: fold matmuls with is_transpose=True -> WRONG RESULTS (rel err
    # 2e5): transpose mode is a different PE datapath, not a GEMM flag.
    "b9t": dict(
        store_mode="fold", split_io=False, split_loads=True, bufs=2,
        t_bufs=2, t2_bufs=2, psum_bufs=4, psum2_bufs=4, bf16=True,
        tfold=True,
    ),
    "b10t": dict(
        store_mode="fold8", split_io=False, split_loads=True, bufs=2,
        t_bufs=2, t2_bufs=2, psum_bufs=4, psum2_bufs=4, bf16=True,
        tfold=True, copy_split=True,
    ),
    # b9 PSUM/SBUF buffer tuning (banks: psum_bufs + psum2_bufs <= 8)
    "b9v": dict(
        store_mode="fold", split_io=False, split_loads=True, bufs=2,
        t_bufs=2, t2_bufs=2, psum_bufs=3, psum2_bufs=5, bf16=True,
    ),
    "b9u": dict(
        store_mode="fold", split_io=False, split_loads=True, bufs=2,
        t_bufs=2, t2_bufs=2, psum_bufs=2, psum2_bufs=6, bf16=True,
    ),
    "b9w": dict(
        store_mode="fold", split_io=False, split_loads=True, bufs=2,
        t_bufs=2, t2_bufs=3, psum_bufs=3, psum2_bufs=5, bf16=True,
    ),
    # b9u + pass-2 copies split DVE/ACT by h parity (full-width copies)
    "b9uc": dict(
        store_mode="fold", split_io=False, split_loads=True, bufs=2,
        t_bufs=2, t2_bufs=2, psum_bufs=2, psum2_bufs=6, bf16=True,
        copy_split=True,
    ),
    # b9u + a third t2 buffer set
    "b9ud": dict(
        store_mode="fold", split_io=False, split_loads=True, bufs=2,
        t_bufs=2, t2_bufs=3, psum_bufs=2, psum2_bufs=6, bf16=True,
    ),
    # b10 rescue: 14944 B descriptors + half-width copies split DVE/ACT +
    # b9ud-style buffer tuning
    "b10c": dict(
        store_mode="fold8", split_io=False, split_loads=True, bufs=2,
        t_bufs=2, t2_bufs=2, psum_bufs=2, psum2_bufs=6, bf16=True,
        copy_split=True,
    ),
    # b9ud + deeper load prefetch
    "b9ud3": dict(
        store_mode="fold", split_io=False, split_loads=True, bufs=3,
        t_bufs=2, t2_bufs=3, psum_bufs=2, psum2_bufs=6, bf16=True,
    ),
    # b9ud + max pass-2 psum buffering
    "b9ue": dict(
        store_mode="fold", split_io=False, split_loads=True, bufs=2,
        t_bufs=2, t2_bufs=3, psum_bufs=1, psum2_bufs=7, bf16=True,
    ),
    # b9ud with ALL stores on the SP ring (ACT only loads + pass-1 copies)
    "b9uf": dict(
        store_mode="fold", split_io=False, split_loads=True, bufs=2,
        t_bufs=2, t2_bufs=3, psum_bufs=2, psum2_bufs=6, bf16=True,
        sync_stores=True,
    ),
    # b9ud + third t_t buffer: rep k+1 pass-1 overlaps rep k pass-2
    "b9ug": dict(
        store_mode="fold", split_io=False, split_loads=True, bufs=2,
        t_bufs=3, t2_bufs=3, psum_bufs=2, psum2_bufs=6, bf16=True,
    ),
    # b9ud + pass-1 copies split DVE/ACT by c parity (halves pass-1 latency)
    "b9uh": dict(
        store_mode="fold", split_io=False, split_loads=True, bufs=2,
        t_bufs=2, t2_bufs=3, psum_bufs=2, psum2_bufs=6, bf16=True,
        pass1_split=True,
    ),
    # b9ud with the 4 h-stores merged into ONE DMA per clip (same 7472 B
    # descriptors, fewer instructions; clip 0 -> SP ring, clip 1 -> ACT)
    "b9um": dict(
        store_mode="fold", split_io=False, split_loads=True, bufs=2,
        t_bufs=2, t2_bufs=3, psum_bufs=2, psum2_bufs=6, bf16=True,
        merge_store=True,
    ),
}


def _build_program(reps: int, variant: str = "v1Lt"):
    from concourse import bass, masks, mybir
    from concourse.tile import TileContext

    cfg = _VARIANTS[variant]
    split_io = cfg["split_io"]
    store_mode = cfg["store_mode"]
    bufs = cfg["bufs"]
    psum_bufs = cfg["psum_bufs"]
    act_stores = cfg.get("act_stores", 0)
    # spread the ACT-ring stores evenly over the 8 (b, h) store slots
    act_slots = set()
    if act_stores:
        stride = (B * H) / act_stores
        act_slots = {int(i * stride + stride / 2) for i in range(act_stores)}

    F32 = mybir.dt.float32
    DT = mybir.dt.bfloat16 if cfg.get("bf16") else F32
    nc = bass.Bass()
    inp = nc.declare_dram_parameter("input", [B, S], DT, isOutput=False)
    outp = nc.declare_dram_parameter("out", [B, FRAME, F], DT, isOutput=True)

    with TileContext(nc) as tc:
        with (
            tc.tile_pool(name="ident_pool", bufs=1) as ipool,
            tc.tile_pool(name="a_pool", bufs=bufs) as apool,
            tc.tile_pool(name="t_pool", bufs=cfg.get("t_bufs", bufs)) as tpool,
            tc.tile_pool(name="t2_pool", bufs=cfg.get("t2_bufs", 2)) as t2pool,
            tc.tile_pool(name="psum_pool", bufs=psum_bufs, space="PSUM") as ppool,
            tc.tile_pool(
                name="psum2_pool", bufs=cfg.get("psum2_bufs", 4), space="PSUM"
            ) as ppool2,
        ):
            ident = ipool.tile([128, 128], DT)
            masks.make_identity(nc, ident[:])

            s_t = None
            if store_mode == "fold":
                # fold matrices: S[m, (j c), q] = 1 iff m == 4*(q - 32c) + j,
                # i.e. lhsT column q picks T row l = 4q + j from c-block
                # rhs t_t[:, c, :] (nonzero only for q in [32c, 32c+32)).
                s_t = ipool.tile([128, 16, 128], DT)
                nc.gpsimd.memset(s_t[:], 0.0)
                for j in range(4):
                    for c in range(4):
                        nc.gpsimd.affine_select(
                            out=s_t[:, 4 * j + c, :],
                            in_=s_t[:, 4 * j + c, :],
                            compare_op=mybir.AluOpType.not_equal,
                            fill=1.0,
                            base=128 * c - j,
                            pattern=[[-4, 128]],
                            channel_multiplier=1,
                        )
            elif store_mode == "fold8":
                # S8[m, (j c), q] = 1 iff 128c + m == (8q + j) mod 512.
                # Two affine pieces: m = 8q + j - 128c (q in [16c, 16c+16))
                # and m = 8q + j - 512 - 128c (q in [64+16c, 64+16c+16));
                # each select keeps prior ones where its condition is false.
                s_t = ipool.tile([128, 32, 128], DT)
                nc.gpsimd.memset(s_t[:], 0.0)
                one = nc.gpsimd.to_reg(1.0)
                for j in range(8):
                    for c in range(4):
                        for base in (128 * c - j, 512 + 128 * c - j):
                            nc.gpsimd.affine_select(
                                out=s_t[:, 4 * j + c, :],
                                in_=s_t[:, 4 * j + c, :],
                                compare_op=mybir.AluOpType.not_equal,
                                fill=one,
                                base=base,
                                pattern=[[-8, 128]],
                                channel_multiplier=1,
                            )

            for _rep in range(reps):
                # loads for both clips upfront (own HWDGE ring via nc.scalar):
                # split at the h8=4 boundary so half-0 transposes start after
                # the first MB.
                a_ts = []
                for b in range(B):
                    a_t = apool.tile([128, G_FULL + 1, HOP], DT, tag="a")
                    a_ts.append(a_t)
                    # rows g = h8*128 + p hold samples 512g .. 512g+512
                    if split_io or cfg.get("split_loads"):
                        nc.scalar.dma_start(
                            out=a_t[:, 0:4, :],
                            in_=inp[b, 0 : 128 * 4 * HOP].rearrange(
                                "(h p c) -> p h c", h=4, p=128, c=HOP
                            ),
                        )
                        nc.scalar.dma_start(
                            out=a_t[:, 4:G_FULL, :],
                            in_=inp[
                                b, 128 * 4 * HOP : 128 * G_FULL * HOP
                            ].rearrange(
                                "(h p c) -> p h c", h=G_FULL - 4, p=128, c=HOP
                            ),
                        )
                    else:
                        nc.scalar.dma_start(
                            out=a_t[:, 0:G_FULL, :],
                            in_=inp[b, 0 : 128 * G_FULL * HOP].rearrange(
                                "(h p c) -> p h c", h=G_FULL, p=128, c=HOP
                            ),
                        )
                    # tail: last 41 rows
                    nc.scalar.dma_start(
                        out=a_t[0:G_TAIL, G_FULL, :],
                        in_=inp[b, 128 * G_FULL * HOP : G * HOP].rearrange(
                            "(p c) -> p c", p=G_TAIL, c=HOP
                        ),
                    )

                for b in range(B):
                    a_t = a_ts[b]
                    if store_mode == "fold8":
                        # pass 1 as in "fold"
                        t_t = tpool.tile([128, 4, G], DT, tag="t")
                        for c in range(4):
                            for half in range(2):
                                ps = ppool.tile([128, 512], DT, tag="ps")
                                glen = 512 if half == 0 else G - 512
                                for k in range(4):
                                    h8 = 4 * half + k
                                    rows = 128 if h8 < G_FULL else G_TAIL
                                    nc.tensor.transpose(
                                        out=ps[:, 128 * k : 128 * k + rows],
                                        in_=a_t[
                                            0:rows, h8, 128 * c : 128 * (c + 1)
                                        ],
                                        identity=ident[0:rows, 0:rows],
                                    )
                                nc.scalar.copy(
                                    out=t_t[:, c, 512 * half : 512 * half + glen],
                                    in_=ps[:, 0:glen],
                                )
                        # pass 2: ps2_j[q, g] = T[(8q + j) % 512, g]
                        if cfg.get("overlap_store"):
                            # fold ONCE into t8o[q, j, g]; per-(tau, q-half)
                            # stores read the h-shifted window [h, h+F)
                            t8o = t2pool.tile([128, 8, G], DT, tag="t8o")
                            for j in range(8):
                                for half in range(2):
                                    ps2 = ppool2.tile(
                                        [128, 512], F32, tag="ps2"
                                    )
                                    glen = 512 if half == 0 else G - 512
                                    for c in range(4):
                                        nc.tensor.matmul(
                                            out=ps2[:, 0:glen],
                                            lhsT=s_t[:, 4 * j + c, :],
                                            rhs=t_t[
                                                :,
                                                c,
                                                512 * half : 512 * half + glen,
                                            ],
                                            start=(c == 0),
                                            stop=(c == 3),
                                        )
                                    nc.vector.tensor_copy(
                                        out=t8o[
                                            :, j, 512 * half : 512 * half + glen
                                        ],
                                        in_=ps2[:, 0:glen],
                                    )
                            for tau in range(2):
                                for qr in range(2):
                                    h = 2 * tau + qr
                                    qs = slice(64 * qr, 64 * (qr + 1))
                                    eng = nc.scalar if tau % 2 else nc.sync
                                    eng.dma_start(
                                        out=outp[
                                            b, 512 * h : 512 * (h + 1), :
                                        ].rearrange(
                                            "(q j) f -> q j f", q=64, j=8
                                        ),
                                        in_=t8o[qs, :, h : h + F],
                                    )
                            continue
                        t8 = [
                            t2pool.tile(
                                [128, 8, F], DT, tag=f"t8_{t}", name=f"t8_{t}"
                            )
                            for t in range(2)
                        ]
                        tfold = cfg.get("tfold")
                        for j in range(8):
                            for half in range(2):
                                ps2 = ppool2.tile(
                                    [128, 512], DT if tfold else F32, tag="ps2"
                                )
                                glen = 512 if half == 0 else G - 512
                                for c in range(4):
                                    nc.tensor.matmul(
                                        out=ps2[:, 0:glen],
                                        lhsT=s_t[:, 4 * j + c, :],
                                        rhs=t_t[
                                            :,
                                            c,
                                            512 * half : 512 * half + glen,
                                        ],
                                        start=(c == 0),
                                        stop=(c == 3),
                                        is_transpose=tfold,
                                    )
                                for tau in range(2):
                                    for qr in range(2):
                                        h = 2 * tau + qr
                                        qs = slice(64 * qr, 64 * (qr + 1))
                                        copy_fn = (
                                            nc.scalar.copy
                                            if cfg.get("copy_split")
                                            and tau != qr
                                            else nc.vector.tensor_copy
                                        )
                                        if half == 0:
                                            copy_fn(
                                                out=t8[tau][qs, j, 0 : 512 - h],
                                                in_=ps2[qs, h:512],
                                            )
                                        else:
                                            ln = min(422 + h, glen)
                                            copy_fn(
                                                out=t8[tau][
                                                    qs, j, 512 - h : 512 - h + ln
                                                ],
                                                in_=ps2[qs, 0:ln],
                                            )
                        for tau in range(2):
                            eng = nc.scalar if tau % 2 else nc.sync
                            eng.dma_start(
                                out=outp[
                                    b, 1024 * tau : 1024 * (tau + 1), :
                                ].rearrange("(q j) f -> q (j f)", q=128, j=8),
                                in_=t8[tau][:, :, :].rearrange(
                                    "p j f -> p (j f)"
                                ),
                            )
                        continue
                    if store_mode == "fold":
                        # pass 1: v1-style transposes -> t_t[p, c, g] =
                        # T[128c + p, g] (ACT does the PSUM->SBUF copies so
                        # DVE is free for the pass-2 copies)
                        t_t = tpool.tile([128, 4, G], DT, tag="t")
                        for c in range(4):
                            p1_copy = (
                                nc.vector.tensor_copy
                                if cfg.get("pass1_split") and c % 2
                                else nc.scalar.copy
                            )
                            for half in range(2):
                                ps = ppool.tile([128, 512], DT, tag="ps")
                                glen = 512 if half == 0 else G - 512  # 425
                                for k in range(4):
                                    h8 = 4 * half + k
                                    rows = 128 if h8 < G_FULL else G_TAIL
                                    nc.tensor.transpose(
                                        out=ps[:, 128 * k : 128 * k + rows],
                                        in_=a_t[
                                            0:rows, h8, 128 * c : 128 * (c + 1)
                                        ],
                                        identity=ident[0:rows, 0:rows],
                                    )
                                p1_copy(
                                    out=t_t[:, c, 512 * half : 512 * half + glen],
                                    in_=ps[:, 0:glen],
                                )
                        # pass 2: fold partitions so partition q holds out
                        # rows 4q+j -> ps2[q, g] = T[4q + j, g]; exactly one
                        # c contributes per (q, g), so bf16 accumulate is
                        # exact.
                        if cfg.get("overlap_store"):
                            # fold ONCE into t4[q, j, g]; the 4 h-stores read
                            # overlapping windows t4[:, :, h:h+F]
                            t4 = t2pool.tile([128, 4, G], DT, tag="t4")
                            for j in range(4):
                                for half in range(2):
                                    ps2 = ppool2.tile(
                                        [128, 512], F32, tag="ps2"
                                    )
                                    glen = 512 if half == 0 else G - 512
                                    for c in range(4):
                                        nc.tensor.matmul(
                                            out=ps2[:, 0:glen],
                                            lhsT=s_t[:, 4 * j + c, :],
                                            rhs=t_t[
                                                :,
                                                c,
                                                512 * half : 512 * half + glen,
                                            ],
                                            start=(c == 0),
                                            stop=(c == 3),
                                        )
                                    nc.vector.tensor_copy(
                                        out=t4[
                                            :, j, 512 * half : 512 * half + glen
                                        ],
                                        in_=ps2[:, 0:glen],
                                    )
                            for h in range(H):
                                eng = nc.scalar if h % 2 else nc.sync
                                eng.dma_start(
                                    out=outp[
                                        b, 512 * h : 512 * (h + 1), :
                                    ].rearrange("(q j) f -> q j f", q=128, j=4),
                                    in_=t4[:, :, h : h + F],
                                )
                            continue
                        merge = cfg.get("merge_store")
                        if merge:
                            t2m = t2pool.tile([128, H, 4, F], DT, tag="t2m")
                        else:
                            t2 = [
                                t2pool.tile(
                                    [128, 4, F],
                                    DT,
                                    tag=f"t2_{h}",
                                    name=f"t2_{h}",
                                )
                                for h in range(H)
                            ]
                        tfold = cfg.get("tfold")
                        for j in range(4):
                            for half in range(2):
                                ps2 = ppool2.tile(
                                    [128, 512], DT if tfold else F32, tag="ps2"
                                )
                                glen = 512 if half == 0 else G - 512
                                n_pass = 2 if cfg.get("pe_x2") else 1
                                for _p in range(n_pass):
                                    for c in range(4):
                                        nc.tensor.matmul(
                                            out=ps2[:, 0:glen],
                                            lhsT=s_t[:, 4 * j + c, :],
                                            rhs=t_t[
                                                :,
                                                c,
                                                512 * half : 512 * half + glen,
                                            ],
                                            start=(c == 0),
                                            stop=(c == 3),
                                            is_transpose=tfold,
                                        )
                                for h in range(H):
                                    copy_fn = (
                                        nc.scalar.copy
                                        if cfg.get("copy_split") and h % 2
                                        else nc.vector.tensor_copy
                                    )
                                    if half == 0:
                                        # f in [0, 512-h) <- g = h + f
                                        copy_fn(
                                            out=(
                                                t2m[:, h, j, 0 : 512 - h]
                                                if merge
                                                else t2[h][:, j, 0 : 512 - h]
                                            ),
                                            in_=ps2[:, h:512],
                                        )
                                    else:
                                        ln = min(422 + h, glen)
                                        copy_fn(
                                            out=(
                                                t2m[
                                                    :, h, j, 512 - h : 512 - h + ln
                                                ]
                                                if merge
                                                else t2[h][
                                                    :, j, 512 - h : 512 - h + ln
                                                ]
                                            ),
                                            in_=ps2[:, 0:ln],
                                        )
                        if merge:
                            eng = nc.sync if b == 0 else nc.scalar
                            eng.dma_start(
                                out=outp[b].rearrange(
                                    "(h q j) f -> q h j f", h=4, q=128, j=4
                                ),
                                in_=t2m[:, :, :, :],
                            )
                            continue
                        for h in range(H):
                            eng = (
                                nc.sync
                                if cfg.get("sync_stores") or h % 2 == 0
                                else nc.scalar
                            )
                            eng.dma_start(
                                out=outp[b, 512 * h : 512 * (h + 1), :].rearrange(
                                    "(q j) f -> q (j f)", q=128, j=4
                                ),
                                in_=t2[h][:, :, :].rearrange("p j f -> p (j f)"),
                            )
                        continue
                    if store_mode == "interleaved":
                        # T2h[q, j, f] = out[b, 512h + 4q + j, f]; per-h tiles
                        # of exactly [128, 4, 934] make (j, f) contiguous per
                        # partition -> 14944 B store descriptors.
                        #
                        # A strided-free-dim f32 lhsT crashes the NC
                        # (NRT_EXEC_UNIT_UNRECOVERABLE, probed in isolation),
                        # so pre-permute columns on ACT: a_perm[p, h8, j, q] =
                        # a_t[p, h8, 4q + j]; every matmul then reads a
                        # contiguous 128-column slice.
                        a_perm = apool.tile(
                            [128, G_FULL + 1, 4, 128], DT, tag="a_perm"
                        )
                        perm_copy = (
                            nc.vector.tensor_copy
                            if cfg.get("dve_permute")
                            else nc.scalar.copy
                        )
                        perm_copy(
                            out=a_perm[:, 0:G_FULL, :, :],
                            in_=a_t[:, 0:G_FULL, :].rearrange(
                                "p h (q j) -> p h j q", q=128, j=4
                            ),
                        )
                        perm_copy(
                            out=a_perm[0:G_TAIL, G_FULL, :, :],
                            in_=a_t[0:G_TAIL, G_FULL, :].rearrange(
                                "p (q j) -> p j q", q=128, j=4
                            ),
                        )
                        t2 = [
                            tpool.tile(
                                [128, 4, F], DT, tag=f"t2_{h}", name=f"t2_{h}"
                            )
                            for h in range(H)
                        ]
                        for j in range(4):
                            for half in range(2):
                                ps = ppool.tile([128, 512], DT, tag="ps")
                                glen = 512 if half == 0 else G - 512  # 425
                                for k in range(4):
                                    h8 = 4 * half + k
                                    rows = 128 if h8 < G_FULL else G_TAIL
                                    nc.tensor.transpose(
                                        out=ps[:, 128 * k : 128 * k + rows],
                                        in_=a_perm[0:rows, h8, j, :],
                                        identity=ident[0:rows, 0:rows],
                                    )
                                # ps[q, col] = T row (4q+j), g = 512*half+col
                                for h in range(H):
                                    if half == 0:
                                        # f in [0, 512-h) <- g = h + f
                                        nc.vector.tensor_copy(
                                            out=t2[h][:, j, 0 : 512 - h],
                                            in_=ps[:, h:512],
                                        )
                                    else:
                                        # f in [512-h, ...) <- g = h + f
                                        ln = min(422 + h, glen)
                                        nc.vector.tensor_copy(
                                            out=t2[h][:, j, 512 - h : 512 - h + ln],
                                            in_=ps[:, 0:ln],
                                        )
                        for h in range(H):
                            eng = (
                                nc.scalar
                                if (b * H + h) in act_slots
                                else nc.sync
                            )
                            eng.dma_start(
                                out=outp[b, 512 * h : 512 * (h + 1), :].rearrange(
                                    "(q j) f -> q (j f)", q=128, j=4
                                ),
                                in_=t2[h][:, :, :].rearrange("p j f -> p (j f)"),
                            )
                        continue
                    if store_mode == "linear2":
                        # timing-only: 2 stores x [128, 7472] = same output
                        # bytes as 14944 B linear descriptors
                        big = tpool.tile([128, 7472], DT, tag="big")
                        nc.vector.memset(big[:, 0:1], 0.0)
                        flat = outp[b].rearrange("w f -> (w f)")
                        n = 128 * 7472
                        for i in range(2):
                            eng = nc.scalar if i % 2 else nc.sync
                            eng.dma_start(
                                out=flat[i * n : (i + 1) * n].rearrange(
                                    "(p q) -> p q", p=128, q=7472
                                ),
                                in_=big[:, :],
                            )
                        continue
                    t_t = tpool.tile([128, 4, G], DT, tag="t")
                    if cfg.get("dma_only"):
                        # give t_t a writer so Tile allocates it
                        nc.vector.memset(t_t[:, 0, 0:1], 0.0)
                    copy_engs = cfg.get("copy_engines", "vvvv")
                    eng_map = {
                        "v": nc.vector.tensor_copy,
                        "a": nc.scalar.copy,
                        "p": nc.gpsimd.tensor_copy,
                    }
                    for c in range(4):
                        if cfg.get("dma_only"):
                            break
                        copy_fn = eng_map[copy_engs[c]]
                        for half in range(2):
                            ps = ppool.tile([128, 512], DT, tag="ps")
                            glen = 512 if half == 0 else G - 512  # 425
                            for k in range(4):
                                h8 = 4 * half + k
                                rows = 128 if h8 < G_FULL else G_TAIL
                                nc.tensor.transpose(
                                    out=ps[:, 128 * k : 128 * k + rows],
                                    in_=a_t[0:rows, h8, 128 * c : 128 * (c + 1)],
                                    identity=ident[0:rows, 0:rows],
                                )
                            copy_fn(
                                out=t_t[:, c, 512 * half : 512 * half + glen],
                                in_=ps[:, 0:glen],
                            )

                    if store_mode == "linear":
                        # timing-only: 4 stores x [128, 3748] covering the
                        # same output bytes with 14992 B linear descriptors
                        flat = outp[b].rearrange("w f -> (w f)")
                        n = 128 * 3736
                        for i in range(4):
                            eng = (
                                nc.scalar
                                if (act_stores and i % 2)
                                else nc.sync
                            )
                            eng.dma_start(
                                out=flat[i * n : (i + 1) * n].rearrange(
                                    "(p q) -> p q", p=128, q=3736
                                ),
                                in_=t_t[:, :, :].rearrange("p c g -> p (c g)")[
                                    :, 0:3736
                                ],
                            )
                        continue
                    for h in cfg.get("store_hs", range(H)):
                        # DRAM rows 512*h + c*128 + p; descriptors are
                        # contiguous 3736 B f-runs either way.
                        if store_mode == "per_c":
                            # one store per c-block: [128, 934], DRAM fully
                            # sequential within the store
                            for c in range(4):
                                nc.sync.dma_start(
                                    out=outp[
                                        b,
                                        512 * h + 128 * c : 512 * h + 128 * (c + 1),
                                        :,
                                    ],
                                    in_=t_t[:, c, h : h + F],
                                )
                            continue
                        dram = outp[b, 512 * h : 512 * (h + 1), :].rearrange(
                            "(c p) f -> p c f", c=4, p=128
                        )
                        if split_io:
                            fsplit = 512 - h
                            nc.sync.dma_start(
                                out=dram[:, :, 0:fsplit],
                                in_=t_t[:, :, h : h + fsplit],
                            )
                            nc.sync.dma_start(
                                out=dram[:, :, fsplit:F],
                                in_=t_t[:, :, 512 : h + F],
                            )
                        else:
                            eng = (
                                nc.scalar
                                if (b * H + h) in act_slots
                                else nc.sync
                            )
                            eng.dma_start(
                                out=dram, in_=t_t[:, :, h : h + F]
                            )

    # TRN2 Matmult (and most instructions) encode at most 1 sync wait; the
    # Tile flow skips the bacc pass that splits extra waits into
    # InstEventSemaphore carriers, so run it here.
    import bass_rust

    bass_rust.generate_event_semaphores(nc)
    return nc


class _Runner:
    """Persistent jitted SPMD runner (modeled on bass2jax.run_bass_via_pjrt,
    but caches the jitted executable across calls).

    donate=False keeps the zero output-donor buffers reusable across calls,
    which lets timing loops run with fully device-resident operands."""

    def __init__(self, reps: int, donate: bool = True, variant: str = "v1Lt"):
        import jax
        from concourse import bass2jax, mybir
        from jax.experimental.shard_map import shard_map
        from jax.sharding import Mesh, PartitionSpec

        bass2jax.install_neuronx_cc_hook()
        self._jax = jax
        nc = _build_program(reps, variant)
        self._nc = nc

        partition_name = (
            nc.partition_id_tensor.name if nc.partition_id_tensor else None
        )
        in_names: list[str] = []
        out_names: list[str] = []
        out_avals = []
        self._zero_shapes = []
        self._in_dtype = np.float32
        for alloc in nc.m.functions[0].allocations:
            if not isinstance(alloc, mybir.MemoryLocationSet):
                continue
            name = alloc.memorylocations[0].name
            if alloc.kind == "ExternalInput":
                if name != partition_name:
                    in_names.append(name)
                    self._in_dtype = mybir.dt.np(alloc.dtype)
            elif alloc.kind == "ExternalOutput":
                out_names.append(name)
                shape = tuple(alloc.tensor_shape)
                dtype = mybir.dt.np(alloc.dtype)
                out_avals.append(jax.core.ShapedArray(shape, dtype))
                self._zero_shapes.append((shape, dtype))
        n_params = len(in_names)
        n_outs = len(out_avals)
        in_names_full = [*in_names, *out_names]
        if partition_name is not None:
            in_names_full.append(partition_name)

        def _body(*args):
            operands = list(args)
            if partition_name is not None:
                operands.append(bass2jax.partition_id_tensor())
            outs = bass2jax._bass_exec_p.bind(
                *operands,
                out_avals=tuple(out_avals),
                in_names=tuple(in_names_full),
                out_names=tuple(out_names),
                lowering_input_output_aliases=(),
                sim_require_finite=True,
                sim_require_nnan=True,
                nc=nc,
            )
            return tuple(outs)

        devices = jax.devices()[:N_CORES]
        assert len(devices) == N_CORES, devices
        mesh = Mesh(np.asarray(devices), ("core",))
        self._mesh = mesh
        self._pspec = PartitionSpec("core")
        donate_argnums = (
            tuple(range(n_params, n_params + n_outs)) if donate else ()
        )
        self._sharded = jax.jit(
            shard_map(
                _body,
                mesh=mesh,
                in_specs=(PartitionSpec("core"),) * (n_params + n_outs),
                out_specs=(PartitionSpec("core"),) * n_outs,
                check_rep=False,
            ),
            donate_argnums=donate_argnums,
            keep_unused=True,
        )

    def fresh_zeros(self):
        return [
            np.zeros((N_CORES * s[0], *s[1:]), d) for s, d in self._zero_shapes
        ]

    def __call__(self, x: np.ndarray, zeros=None):
        # shard_map splits axis 0 across the 8 cores: rows [2i, 2i+2) land on
        # core i — exactly the batch sharding. Global in/out pass through.
        if zeros is None:
            zeros = self.fresh_zeros()
        out = self._sharded(np.asarray(x, dtype=self._in_dtype), *zeros)[0]
        return np.asarray(out)

    def device_args(self, x: np.ndarray):
        """device_put the operands once, sharded over the mesh."""
        import jax
        from jax.sharding import NamedSharding

        sh = NamedSharding(self._mesh, self._pspec)
        x = np.asarray(x, dtype=self._in_dtype)
        return [jax.device_put(a, sh) for a in (x, *self.fresh_zeros())]

    def dispatch(self, args):
        """Launch without fetching results; returns device array handles."""
        return self._sharded(*args)


import os as _os

DEFAULT_VARIANT = _os.environ.get("ENFRAME_VARIANT", "b9um")


def get_runner(
    reps: int = 1, donate: bool = True, variant: str = DEFAULT_VARIANT
) -> "_Runner":
    key = ("runner", reps, donate, variant)
    if key not in _CACHE:
        _CACHE[key] = _Runner(reps, donate, variant)
    return _CACHE[key]


def kernel(input: np.ndarray) -> np.ndarray:
    x = np.ascontiguousarray(input, dtype=np.float32)
    assert x.shape == (BATCH, S), x.shape
    out = get_runner(1)(x)
    return np.asarray(out).astype(np.float32)

